# revision 5
# baseline (speedup 1.0000x reference)
"""LoMoE output head kernel for 8 Trainium2 NeuronCores.

Strategy (tensor-parallel over in_features):
  - The dominant cost is streaming x [32,21,512,64] (88 MB f32) and the
    projection weights through HBM.  We shard the 32768-long feature axis
    8 ways: core k owns features [4096k, 4096(k+1)).
  - Each core computes a partial [160, 672] = [W_base | lora_A_flat] @ x_shard.T
    in bf16 (f32 PSUM accumulation).  That one fused matmul covers both the
    base projection (96 rows) and all-expert LoRA "temp" (64 = 8 experts x
    rank 8 rows).
  - Host gathers the 8 partials, sums them (the TP all-reduce), applies the
    router (top-2 softmax gating, computed on host from the pooled means),
    the rank-8 expert combine, and the bias.
"""

import numpy as np
import ml_dtypes

import concourse.bass as bass
import concourse.mybir as mybir
import concourse.tile as tile
from concourse import bacc
from concourse.bass_utils import run_bass_kernel_spmd

B, V, D, P = 32, 21, 512, 64
T = B * V                  # 672 tokens
IN = D * P                 # 32768
OUT = 96
E, RK = 8, 8
M2 = E * RK                # 64 lora rows
MTOT = OUT + M2            # 160
NC = 8                     # cores
F = IN // NC               # 4096 features per core
CH = F // 128              # 32 K-chunks of 128
TOP_K = 2
SCALING = 16.0 / 8.0

NGRP = 4                   # x/w are DMA'd in NGRP chunk-groups for pipelining
CPG = CH // NGRP           # K-chunks per group
NT = 2                     # token tiles (PSUM bank holds <=512 f32)
TT = T // NT               # 336

BF16 = mybir.dt.bfloat16
F32 = mybir.dt.float32
np_bf16 = ml_dtypes.bfloat16


def _build_bass():
    nc = bacc.Bacc("TRN2", target_bir_lowering=False, debug=False)
    # x_sb[p, c, t] = x_flat[t, shard_base + c*128 + p]   (bf16)
    x_d = nc.dram_tensor("x_sb", [128, CH, T], BF16, kind="ExternalInput")
    # w_sb[p, c, m] = Wcat[m, shard_base + c*128 + p]     (bf16)
    w_d = nc.dram_tensor("w_sb", [128, CH, MTOT], BF16, kind="ExternalInput")
    o_d = nc.dram_tensor("out_part", [MTOT, T], F32, kind="ExternalOutput")

    with tile.TileContext(nc) as tc:
        with (
            tc.tile_pool(name="wp", bufs=1) as wp,
            tc.tile_pool(name="xp", bufs=1) as xp,
            tc.tile_pool(name="pp", bufs=1, space="PSUM") as pp,
            tc.tile_pool(name="op", bufs=1) as op,
        ):
            w_tiles = []
            x_tiles = []
            for g in range(NGRP):
                wt = wp.tile([128, CPG, MTOT], BF16, name=f"w{g}", tag=f"w{g}")
                nc.sync.dma_start(wt[:], w_d[:, g * CPG:(g + 1) * CPG, :])
                xt = xp.tile([128, CPG, T], BF16, name=f"x{g}", tag=f"x{g}")
                nc.sync.dma_start(xt[:], x_d[:, g * CPG:(g + 1) * CPG, :])
                w_tiles.append(wt)
                x_tiles.append(xt)

            psB = [pp.tile([OUT, TT], F32, name=f"psB{j}", tag=f"psB{j}") for j in range(NT)]
            psL = [pp.tile([M2, TT], F32, name=f"psL{j}", tag=f"psL{j}") for j in range(NT)]

            for g in range(NGRP):
                wt, xt = w_tiles[g], x_tiles[g]
                for cl in range(CPG):
                    c = g * CPG + cl
                    first = c == 0
                    last = c == CH - 1
                    for j in range(NT):
                        nc.tensor.matmul(
                            psB[j][:],
                            wt[:, cl, 0:OUT],
                            xt[:, cl, j * TT:(j + 1) * TT],
                            start=first,
                            stop=last,
                        )
                    for j in range(NT):
                        nc.tensor.matmul(
                            psL[j][:],
                            wt[:, cl, OUT:MTOT],
                            xt[:, cl, j * TT:(j + 1) * TT],
                            start=first,
                            stop=last,
                        )

            ob = op.tile([OUT, T], F32, name="ob", tag="ob")
            ol = op.tile([M2, T], F32, name="ol", tag="ol")
            for j in range(NT):
                nc.vector.tensor_copy(ob[:, j * TT:(j + 1) * TT], psB[j][:])
                nc.vector.tensor_copy(ol[:, j * TT:(j + 1) * TT], psL[j][:])
            nc.sync.dma_start(o_d[0:OUT, :], ob[:])
            nc.sync.dma_start(o_d[OUT:MTOT, :], ol[:])

    nc.compile()
    return nc


def _host_router(x, W1, b1, W2, b2):
    """Top-2 softmax gating, computed exactly (f64) on the pooled means."""
    pooled = x.astype(np.float64).mean(axis=(1, 3))            # [B, D]
    h = np.maximum(pooled @ W1.astype(np.float64).T + b1, 0.0)
    logits = h @ W2.astype(np.float64).T + b2
    z = np.exp(logits - logits.max(-1, keepdims=True))
    probs = z / z.sum(-1, keepdims=True)
    topi = np.argsort(-probs, axis=-1, kind="stable")[:, :TOP_K]
    topw = np.take_along_axis(probs, topi, axis=-1)
    topw = topw / np.clip(topw.sum(-1, keepdims=True), 1e-6, None)
    w_full = np.zeros((B, E))
    np.put_along_axis(w_full, topi, topw, axis=-1)
    return probs.astype(np.float32), w_full


def _run(inputs, trace=False):
    x = np.asarray(inputs["x"], dtype=np.float32)
    W_base = np.asarray(inputs["W_base"], dtype=np.float32)
    b_base = np.asarray(inputs["b_base"], dtype=np.float32)
    lora_A = np.asarray(inputs["lora_A"], dtype=np.float32)
    lora_B = np.asarray(inputs["lora_B"], dtype=np.float32)

    flat = x.reshape(T, IN)
    probs, w_full = _host_router(
        x,
        np.asarray(inputs["W1"], dtype=np.float32),
        np.asarray(inputs["b1"], dtype=np.float32),
        np.asarray(inputs["W2"], dtype=np.float32),
        np.asarray(inputs["b2"], dtype=np.float32),
    )

    Wcat = np.concatenate([W_base, lora_A.reshape(M2, IN)], axis=0)  # [160, IN]

    in_maps = []
    for k in range(NC):
        sl = slice(k * F, (k + 1) * F)
        # [F, T] -> [CH, 128, T] -> [128, CH, T]
        xsb = np.ascontiguousarray(
            flat[:, sl].T.reshape(CH, 128, T).transpose(1, 0, 2)
        ).astype(np_bf16)
        wsb = np.ascontiguousarray(
            Wcat[:, sl].T.reshape(CH, 128, MTOT).transpose(1, 0, 2)
        ).astype(np_bf16)
        in_maps.append({"x_sb": xsb, "w_sb": wsb})

    nc = _build_bass()
    res = run_bass_kernel_spmd(nc, in_maps, core_ids=list(range(NC)), trace=trace)

    total = np.zeros((MTOT, T), np.float32)
    for r in res.results:
        total += r["out_part"]

    base = total[:OUT].T + b_base                               # [T, 96]
    temp = total[OUT:]                                          # [64, T]
    # Mb[b] @ temp[:, tokens of b] folds lora_B, gate weight and SCALING.
    Mb = SCALING * (w_full[:, :, None, None] * lora_B[None])    # [B, E, 96, R]
    Mb = np.transpose(Mb, (0, 2, 1, 3)).reshape(B, OUT, M2).astype(np.float32)
    out = np.empty((B, V, OUT), np.float32)
    for b in range(B):
        tb = temp[:, b * V:(b + 1) * V]
        out[b] = base[b * V:(b + 1) * V] + (Mb[b] @ tb).T
    return out, probs, res


def kernel(**inputs):
    out, probs, _ = _run(inputs)
    return out, probs


# revision 7
# speedup vs baseline: 1.0446x; 1.0446x over previous
"""LoMoE output head kernel for 8 Trainium2 NeuronCores.

Strategy (tensor-parallel over in_features):
  - The dominant cost is streaming x [32,21,512,64] (88 MB f32) and the
    projection weights through HBM.  We shard the 32768-long feature axis
    8 ways: core k owns features [4096k, 4096(k+1)).
  - Each core computes a partial [160, 672] = [W_base | lora_A_flat] @ x_shard.T
    in bf16 (f32 PSUM accumulation).  That one fused matmul covers both the
    base projection (96 rows) and all-expert LoRA "temp" (64 = 8 experts x
    rank 8 rows).
  - Host gathers the 8 partials, sums them (the TP all-reduce), applies the
    router (top-2 softmax gating, computed on host from the pooled means),
    the rank-8 expert combine, and the bias.
"""

import numpy as np
import ml_dtypes

import concourse.bass as bass
import concourse.mybir as mybir
import concourse.tile as tile
from concourse import bacc
from concourse.bass_utils import run_bass_kernel_spmd

B, V, D, P = 32, 21, 512, 64
T = B * V                  # 672 tokens
IN = D * P                 # 32768
OUT = 96
E, RK = 8, 8
M2 = E * RK                # 64 lora rows
MTOT = OUT + M2            # 160
NC = 8                     # cores
F = IN // NC               # 4096 features per core
CH = F // 128              # 32 K-chunks of 128
TOP_K = 2
SCALING = 16.0 / 8.0

# x/w are DMA'd in progressive chunk-groups: small first groups so the PE
# starts early, large later groups for DMA efficiency.
GRPS = [2, 2, 4, 8, 8, 8]
assert sum(GRPS) == CH
NT = 2                     # token tiles (PSUM bank holds <=512 f32)
TT = T // NT               # 336
M1 = 128                   # stationary 1: base(96) + lora rows 0:32  (FWL-wide)
MR = MTOT - M1             # stationary 2: lora rows 32:64

BF16 = mybir.dt.bfloat16
F32 = mybir.dt.float32
np_bf16 = ml_dtypes.bfloat16


def _build_bass():
    nc = bacc.Bacc("TRN2", target_bir_lowering=False, debug=False)
    # x_sb[p, c, t] = x_flat[t, shard_base + c*128 + p]   (bf16)
    x_d = nc.dram_tensor("x_sb", [128, CH, T], BF16, kind="ExternalInput")
    # w_sb[p, c, m] = Wcat[m, shard_base + c*128 + p]     (bf16)
    w_d = nc.dram_tensor("w_sb", [128, CH, MTOT], BF16, kind="ExternalInput")
    o_d = nc.dram_tensor("out_part", [MTOT, T], F32, kind="ExternalOutput")

    with tile.TileContext(nc) as tc:
        with (
            tc.tile_pool(name="wp", bufs=1) as wp,
            tc.tile_pool(name="xp", bufs=1) as xp,
            tc.tile_pool(name="pp", bufs=1, space="PSUM") as pp,
            tc.tile_pool(name="op", bufs=1) as op,
        ):
            w_tiles = []
            x_tiles = []
            c0 = 0
            for g, cpg in enumerate(GRPS):
                wt = wp.tile([128, cpg, MTOT], BF16, name=f"w{g}", tag=f"w{g}")
                nc.sync.dma_start(wt[:], w_d[:, c0:c0 + cpg, :])
                xt = xp.tile([128, cpg, T], BF16, name=f"x{g}", tag=f"x{g}")
                nc.sync.dma_start(xt[:], x_d[:, c0:c0 + cpg, :])
                w_tiles.append(wt)
                x_tiles.append(xt)
                c0 += cpg

            psA = [pp.tile([M1, TT], F32, name=f"psA{j}", tag=f"psA{j}") for j in range(NT)]
            psR = [pp.tile([MR, TT], F32, name=f"psR{j}", tag=f"psR{j}") for j in range(NT)]

            c0 = 0
            for g, cpg in enumerate(GRPS):
                wt, xt = w_tiles[g], x_tiles[g]
                for cl in range(cpg):
                    c = c0 + cl
                    first = c == 0
                    last = c == CH - 1
                    for j in range(NT):
                        nc.tensor.matmul(
                            psA[j][:],
                            wt[:, cl, 0:M1],
                            xt[:, cl, j * TT:(j + 1) * TT],
                            start=first,
                            stop=last,
                        )
                    for j in range(NT):
                        nc.tensor.matmul(
                            psR[j][:],
                            wt[:, cl, M1:MTOT],
                            xt[:, cl, j * TT:(j + 1) * TT],
                            start=first,
                            stop=last,
                        )
                c0 += cpg

            ob = op.tile([M1, T], F32, name="ob", tag="ob")
            ol = op.tile([MR, T], F32, name="ol", tag="ol")
            for j in range(NT):
                nc.vector.tensor_copy(ob[:, j * TT:(j + 1) * TT], psA[j][:])
                nc.vector.tensor_copy(ol[:, j * TT:(j + 1) * TT], psR[j][:])
            nc.sync.dma_start(o_d[0:M1, :], ob[:])
            nc.sync.dma_start(o_d[M1:MTOT, :], ol[:])

    nc.compile()
    return nc


def _host_router(x, W1, b1, W2, b2):
    """Top-2 softmax gating, computed exactly (f64) on the pooled means."""
    pooled = x.astype(np.float64).mean(axis=(1, 3))            # [B, D]
    h = np.maximum(pooled @ W1.astype(np.float64).T + b1, 0.0)
    logits = h @ W2.astype(np.float64).T + b2
    z = np.exp(logits - logits.max(-1, keepdims=True))
    probs = z / z.sum(-1, keepdims=True)
    topi = np.argsort(-probs, axis=-1, kind="stable")[:, :TOP_K]
    topw = np.take_along_axis(probs, topi, axis=-1)
    topw = topw / np.clip(topw.sum(-1, keepdims=True), 1e-6, None)
    w_full = np.zeros((B, E))
    np.put_along_axis(w_full, topi, topw, axis=-1)
    return probs.astype(np.float32), w_full


def _run(inputs, trace=False):
    x = np.asarray(inputs["x"], dtype=np.float32)
    W_base = np.asarray(inputs["W_base"], dtype=np.float32)
    b_base = np.asarray(inputs["b_base"], dtype=np.float32)
    lora_A = np.asarray(inputs["lora_A"], dtype=np.float32)
    lora_B = np.asarray(inputs["lora_B"], dtype=np.float32)

    flat = x.reshape(T, IN)
    probs, w_full = _host_router(
        x,
        np.asarray(inputs["W1"], dtype=np.float32),
        np.asarray(inputs["b1"], dtype=np.float32),
        np.asarray(inputs["W2"], dtype=np.float32),
        np.asarray(inputs["b2"], dtype=np.float32),
    )

    Wcat = np.concatenate([W_base, lora_A.reshape(M2, IN)], axis=0)  # [160, IN]

    in_maps = []
    for k in range(NC):
        sl = slice(k * F, (k + 1) * F)
        # [F, T] -> [CH, 128, T] -> [128, CH, T]
        xsb = np.ascontiguousarray(
            flat[:, sl].T.reshape(CH, 128, T).transpose(1, 0, 2)
        ).astype(np_bf16)
        wsb = np.ascontiguousarray(
            Wcat[:, sl].T.reshape(CH, 128, MTOT).transpose(1, 0, 2)
        ).astype(np_bf16)
        in_maps.append({"x_sb": xsb, "w_sb": wsb})

    nc = _build_bass()
    res = run_bass_kernel_spmd(nc, in_maps, core_ids=list(range(NC)), trace=trace)

    total = np.zeros((MTOT, T), np.float32)
    for r in res.results:
        total += r["out_part"]

    base = total[:OUT].T + b_base                               # [T, 96]
    temp = total[OUT:]                                          # [64, T]
    # Mb[b] @ temp[:, tokens of b] folds lora_B, gate weight and SCALING.
    Mb = SCALING * (w_full[:, :, None, None] * lora_B[None])    # [B, E, 96, R]
    Mb = np.transpose(Mb, (0, 2, 1, 3)).reshape(B, OUT, M2).astype(np.float32)
    out = np.empty((B, V, OUT), np.float32)
    for b in range(B):
        tb = temp[:, b * V:(b + 1) * V]
        out[b] = base[b * V:(b + 1) * V] + (Mb[b] @ tb).T
    return out, probs, res


def kernel(**inputs):
    out, probs, _ = _run(inputs)
    return out, probs


# revision 12
# speedup vs baseline: 1.1159x; 1.0683x over previous
"""LoMoE output head kernel for 8 Trainium2 NeuronCores.

Strategy (tensor-parallel over in_features):
  - The dominant cost is streaming x [32,21,512,64] (88 MB f32) and the
    projection weights through HBM.  We shard the 32768-long feature axis
    8 ways: core k owns features [4096k, 4096(k+1)).
  - Each core computes a partial [160, 672] = [W_base | lora_A_flat] @ x_shard.T
    in bf16 (f32 PSUM accumulation).  That one fused matmul covers both the
    base projection (96 rows) and all-expert LoRA "temp" (64 = 8 experts x
    rank 8 rows).
  - Host gathers the 8 partials, sums them (the TP all-reduce), applies the
    router (top-2 softmax gating, computed on host from the pooled means),
    the rank-8 expert combine, and the bias.
"""

import numpy as np
import ml_dtypes

import concourse.bass as bass
import concourse.mybir as mybir
import concourse.tile as tile
from concourse import bacc
from concourse.bass_utils import run_bass_kernel_spmd

B, V, D, P = 32, 21, 512, 64
T = B * V                  # 672 tokens
IN = D * P                 # 32768
OUT = 96
E, RK = 8, 8
M2 = E * RK                # 64 lora rows
MTOT = OUT + M2            # 160
NC = 8                     # cores
F = IN // NC               # 4096 features per core
CH = F // 128              # 32 K-chunks of 128
TOP_K = 2
SCALING = 16.0 / 8.0

# x/w are DMA'd in progressive chunk-groups: small first groups so the PE
# starts early, large later groups for DMA efficiency.
GRPS = [2, 2, 4, 8, 8, 8]
assert sum(GRPS) == CH
NT = 2                     # token tiles (PSUM bank holds <=512 f32)
TT = T // NT               # 336

BF16 = mybir.dt.bfloat16
F32 = mybir.dt.float32
np_bf16 = ml_dtypes.bfloat16


def _build_bass():
    nc = bacc.Bacc("TRN2", target_bir_lowering=False, debug=False)
    # x_sb[p, c, t] = x_flat[t, shard_base + c*128 + p]   (bf16)
    x_d = nc.dram_tensor("x_sb", [128, CH, T], BF16, kind="ExternalInput")
    # w_sb[p, c, m] = Wcat[m, shard_base + c*128 + p]     (bf16)
    w_d = nc.dram_tensor("w_sb", [128, CH, MTOT], BF16, kind="ExternalInput")
    # rows 0:96 base, 96:160 lora partial (even chunks), 160:224 lora partial
    # (odd chunks) -- host adds the two lora partials.
    o_d = nc.dram_tensor("out_part", [OUT + 2 * M2, T], F32, kind="ExternalOutput")

    with tile.TileContext(nc) as tc:
        with (
            tc.tile_pool(name="wp", bufs=1) as wp,
            tc.tile_pool(name="xp", bufs=1) as xp,
            tc.tile_pool(name="pp", bufs=1, space="PSUM") as pp,
            tc.tile_pool(name="op", bufs=1) as op,
        ):
            w_tiles = []
            x_tiles = []
            c0 = 0
            for g, cpg in enumerate(GRPS):
                # w on the scalar HWDGE ring, x on the sync ring: the two
                # streams flow in parallel so the first matmul isn't gated on
                # a serial w-then-x chain.
                wt = wp.tile([128, cpg, MTOT], BF16, name=f"w{g}", tag=f"w{g}")
                nc.scalar.dma_start(wt[:], w_d[:, c0:c0 + cpg, :])
                xt = xp.tile([128, cpg, T], BF16, name=f"x{g}", tag=f"x{g}")
                nc.sync.dma_start(xt[:], x_d[:, c0:c0 + cpg, :])
                w_tiles.append(wt)
                x_tiles.append(xt)
                c0 += cpg

            # base accumulators: [96, TT] x2; lora accumulators: even chunks
            # at PSUM partitions 0:64, odd chunks at partitions 64:128 (col
            # tiling lets the even/odd lora matmuls of a chunk pair stream
            # concurrently through disjoint column groups of the PE array).
            psB = [pp.tile([OUT, TT], F32, name=f"psB{j}", tag=f"psB{j}") for j in range(NT)]
            psL = [pp.tile([128, TT], F32, name=f"psL{j}", tag=f"psL{j}") for j in range(NT)]

            def chunk_tiles(c):
                cg, cl = 0, c
                for g, cpg in enumerate(GRPS):
                    if cl < cpg:
                        return w_tiles[g], x_tiles[g], cl
                    cl -= cpg
                raise AssertionError

            for c2 in range(CH // 2):
                ce, co = 2 * c2, 2 * c2 + 1
                wte, xte, cle = chunk_tiles(ce)
                wto, xto, clo = chunk_tiles(co)
                first = c2 == 0
                last = c2 == CH // 2 - 1
                # two base passes (96 cols each)
                for idx, (wt_, xt_, cl_) in enumerate(((wte, xte, cle), (wto, xto, clo))):
                    for j in range(NT):
                        nc.tensor.matmul(
                            psB[j][:],
                            wt_[:, cl_, 0:OUT],
                            xt_[:, cl_, j * TT:(j + 1) * TT],
                            start=first and idx == 0,
                            stop=last and idx == 1,
                        )
                # one packed lora pass: even chunk -> cols/partitions 0:64,
                # odd chunk -> cols/partitions 64:128, concurrent streams.
                for j in range(NT):
                    nc.tensor.matmul(
                        psL[j][0:M2, :],
                        wte[:, cle, OUT:MTOT],
                        xte[:, cle, j * TT:(j + 1) * TT],
                        start=first,
                        stop=last,
                    )
                    nc.tensor.matmul(
                        psL[j][M2:128, :],
                        wto[:, clo, OUT:MTOT],
                        xto[:, clo, j * TT:(j + 1) * TT],
                        start=first,
                        stop=last,
                    )

            ob = op.tile([OUT, T], F32, name="ob", tag="ob")
            ol = op.tile([128, T], F32, name="ol", tag="ol")
            for j in range(NT):
                nc.vector.tensor_copy(ob[:, j * TT:(j + 1) * TT], psB[j][:])
                nc.vector.tensor_copy(ol[:, j * TT:(j + 1) * TT], psL[j][:])
            nc.sync.dma_start(o_d[0:OUT, :], ob[:])
            nc.sync.dma_start(o_d[OUT:OUT + 2 * M2, :], ol[:])

    nc.compile()
    return nc


def _host_router(x, W1, b1, W2, b2):
    """Top-2 softmax gating, computed exactly (f64) on the pooled means."""
    pooled = x.astype(np.float64).mean(axis=(1, 3))            # [B, D]
    h = np.maximum(pooled @ W1.astype(np.float64).T + b1, 0.0)
    logits = h @ W2.astype(np.float64).T + b2
    z = np.exp(logits - logits.max(-1, keepdims=True))
    probs = z / z.sum(-1, keepdims=True)
    topi = np.argsort(-probs, axis=-1, kind="stable")[:, :TOP_K]
    topw = np.take_along_axis(probs, topi, axis=-1)
    topw = topw / np.clip(topw.sum(-1, keepdims=True), 1e-6, None)
    w_full = np.zeros((B, E))
    np.put_along_axis(w_full, topi, topw, axis=-1)
    return probs.astype(np.float32), w_full


def _run(inputs, trace=False):
    x = np.asarray(inputs["x"], dtype=np.float32)
    W_base = np.asarray(inputs["W_base"], dtype=np.float32)
    b_base = np.asarray(inputs["b_base"], dtype=np.float32)
    lora_A = np.asarray(inputs["lora_A"], dtype=np.float32)
    lora_B = np.asarray(inputs["lora_B"], dtype=np.float32)

    flat = x.reshape(T, IN)
    probs, w_full = _host_router(
        x,
        np.asarray(inputs["W1"], dtype=np.float32),
        np.asarray(inputs["b1"], dtype=np.float32),
        np.asarray(inputs["W2"], dtype=np.float32),
        np.asarray(inputs["b2"], dtype=np.float32),
    )

    Wcat = np.concatenate([W_base, lora_A.reshape(M2, IN)], axis=0)  # [160, IN]

    in_maps = []
    for k in range(NC):
        sl = slice(k * F, (k + 1) * F)
        # [F, T] -> [CH, 128, T] -> [128, CH, T]
        xsb = np.ascontiguousarray(
            flat[:, sl].T.reshape(CH, 128, T).transpose(1, 0, 2)
        ).astype(np_bf16)
        wsb = np.ascontiguousarray(
            Wcat[:, sl].T.reshape(CH, 128, MTOT).transpose(1, 0, 2)
        ).astype(np_bf16)
        in_maps.append({"x_sb": xsb, "w_sb": wsb})

    nc = _build_bass()
    res = run_bass_kernel_spmd(nc, in_maps, core_ids=list(range(NC)), trace=trace)

    total = np.zeros((OUT + 2 * M2, T), np.float32)
    for r in res.results:
        total += r["out_part"]

    base = total[:OUT].T + b_base                               # [T, 96]
    temp = total[OUT:OUT + M2] + total[OUT + M2:]               # [64, T]
    # Mb[b] @ temp[:, tokens of b] folds lora_B, gate weight and SCALING.
    Mb = SCALING * (w_full[:, :, None, None] * lora_B[None])    # [B, E, 96, R]
    Mb = np.transpose(Mb, (0, 2, 1, 3)).reshape(B, OUT, M2).astype(np.float32)
    out = np.empty((B, V, OUT), np.float32)
    for b in range(B):
        tb = temp[:, b * V:(b + 1) * V]
        out[b] = base[b * V:(b + 1) * V] + (Mb[b] @ tb).T
    return out, probs, res


def kernel(**inputs):
    out, probs, _ = _run(inputs)
    return out, probs
